# revision 20
# baseline (speedup 1.0000x reference)
"""Bahdanau attention Trainium2 kernel (v2).

B=32, T=1, S=4096, H=1024. Data-parallel over batch across 8 NeuronCores
(4 batches/core). Per core, a single-pass streaming kernel over 128
global s-tiles (4 batches x 32 tiles), fully software-pipelined across
batch boundaries:

  - encoder streams in as half-chunks [128s, 2, 1024h] via SWDGE cast-DMA
    (fp32->bf16); tile st covers source rows [st*128, (st+1)*128) with
    partition p <-> row st*128+p
  - fp8 cast runs BEFORE the transpose: each s-tile is cast bf16->fp8
    with pair-interleaved byte layout (h and h+128 adjacent) into a
    [128, 512] bf16-container tile, so the xbar transpose moves HALF the
    bytes of the baseline (the transposed tile is consumed directly as
    DoubleRow fp8 stationary via a bitcast + stride-2 access pattern)
  - cast pieces: r=0 on vector, r=1 on scalar, 4 tiles ahead; transposes
    3 tiles ahead on the serialized sync queue
  - TensorE: h_proj fp8 DoubleRow (8 instrs/tile), inline ctx rank-1
    matmuls trailing 6 tiles (bf16, from the s-major bf16 encoder)
  - VectorE: +q_proj broadcast add
  - ScalarE: tanh; tile-pair exp with free-dim accumulation (softmax
    denominator)
  - GpSimd: the score dot (fused multiply-reduce against v), plus the
    enc-load descriptor generation and small bounces
  - q_proj broadcasts for ALL batches are built in the prologue (one
    DRAM bounce + partition_broadcast each), so batch boundaries do not
    stall; per-batch ctx rows go to SBUF via tiny sync-queue DMAs

softmax is computed without max-subtraction: |score| <= ||v||_1 ~ 26, so
exp stays comfortably inside fp32/bf16 range. Context is accumulated
unnormalized and scaled by 1/denom at batch end.

src_lengths is (faithfully to the reference) unused.
"""
import numpy as np
from contextlib import ExitStack

import concourse.bass as bass
import concourse.tile as tile
from concourse import bacc, mybir, masks
from concourse import bass_isa
from concourse import bass_utils

F32 = mybir.dt.float32
BF16 = mybir.dt.bfloat16
FP8 = mybir.dt.float8e4
Tanh = mybir.ActivationFunctionType.Tanh
Exp = mybir.ActivationFunctionType.Exp
Copy = mybir.ActivationFunctionType.Copy
DR = mybir.MatmulPerfMode.DoubleRow
DRSW = mybir.MatmulPerfMode.DoubleRowSwInterleave
# True: fp8 pack-cast BEFORE the xbar transpose (half the transpose bytes),
# consumed via DoubleRowSwInterleave. False: baseline-style bf16 transpose
# followed by an fp8 cast, consumed via plain DoubleRow.
PACKED = True

B, T, S, H = 32, 1, 4096, 1024
NCORES = 8
BL = B // NCORES       # batches per core
NS = S // 128          # s-tiles per batch
NHB = H // 128         # h blocks
NKB = 2 * H // 128     # k blocks of cat=[ctx;query]
GT = BL * NS           # global s-tiles per core
NHC = S // 256         # half-chunks (2 tiles) per batch
GHC = BL * NHC         # global half-chunks
PREFETCH_HC = 7        # half-chunks the enc DMA runs ahead
CAST_AHEAD = 8         # tiles the fp8 pack-cast runs ahead
TR_AHEAD = 6           # tiles the transpose runs ahead
CTX_DELAY = 6          # tiles the inline ctx matmuls trail


def _build_program():
    nc = bacc.Bacc("TRN2", target_bir_lowering=False, debug=False)

    q_d = nc.dram_tensor("query", (BL, T, H), F32, kind="ExternalInput").ap()
    enc_d = nc.dram_tensor("encoder_outputs", (BL, S, H), F32,
                           kind="ExternalInput").ap()
    ws_d = nc.dram_tensor("Ws_w", (H, H), F32, kind="ExternalInput").ap()
    wh_d = nc.dram_tensor("Wh_w", (H, H), F32, kind="ExternalInput").ap()
    v_d = nc.dram_tensor("v_w", (1, H), F32, kind="ExternalInput").ap()
    wout_d = nc.dram_tensor("Wout_w", (H, 2 * H), F32, kind="ExternalInput").ap()
    out_d = nc.dram_tensor("out", (BL, T, H), F32, kind="ExternalOutput").ap()
    qp_stash = nc.dram_tensor("qp_stash", (1, BL * H), BF16, kind="Internal").ap()

    with tile.TileContext(nc) as tc, ExitStack() as ctx:
        # ---------------- pools ----------------
        wt_pool = ctx.enter_context(tc.tile_pool(name="wt", bufs=1))
        wnat_pool = ctx.enter_context(tc.tile_pool(name="wnat", bufs=4))
        wonat_pool = ctx.enter_context(tc.tile_pool(name="wonat", bufs=2))
        enc_pool = ctx.enter_context(tc.tile_pool(name="encp", bufs=12))
        pk_pool = ctx.enter_context(tc.tile_pool(name="pkp", bufs=12))
        tr_pool = ctx.enter_context(tc.tile_pool(name="trp", bufs=9))
        sum_pool = ctx.enter_context(tc.tile_pool(name="sump", bufs=3))
        tanh_pool = ctx.enter_context(tc.tile_pool(name="tanhp", bufs=4))
        junk_pool = ctx.enter_context(tc.tile_pool(name="junkp", bufs=1))
        sc_pool = ctx.enter_context(tc.tile_pool(name="scp", bufs=2))
        small_pool = ctx.enter_context(tc.tile_pool(name="smallp", bufs=1))

        hp_psum = ctx.enter_context(tc.tile_pool(name="hp_ps", bufs=3, space="PSUM"))
        ctx_psum = ctx.enter_context(tc.tile_pool(name="ctx_ps", bufs=1, space="PSUM"))
        stg_psum = ctx.enter_context(tc.tile_pool(name="stg_ps", bufs=1, space="PSUM"))
        tr_psum = stg_psum  # prologue transpose staging (1 bank)

        # ---------------- constants ----------------
        id128 = small_pool.tile([128, 128], BF16)
        masks.make_identity(nc, id128[:])
        id4 = small_pool.tile([4, 4], BF16)
        masks.make_identity(nc, id4[:])
        ones_col = small_pool.tile([128, 1], BF16)
        nc.gpsimd.memset(ones_col[:], 1.0)

        # ---------------- encoder pipeline helpers ----------------
        hc_tiles = {}

        def hc_dma(ghc):
            """Load half-chunk ghc: [128, 2, 1024] bf16 (cast from fp32)."""
            if ghc in hc_tiles:
                return hc_tiles[ghc]
            b, hc = divmod(ghc, NHC)
            encN = enc_pool.tile([128, 2, H], BF16, tag="encN")
            src = enc_d[b, hc * 256:(hc + 1) * 256, :]
            nc.gpsimd.dma_start(encN[:], src.rearrange("(t p) h -> p t h", p=128))
            hc_tiles[ghc] = encN
            return encN

        def enc_tile(g):
            return hc_dma(g // 2)[:, g % 2, :]

        pk_tiles = {}
        tr_tiles = {}
        trf_tiles = {}

        if PACKED:
            def cast_tile(g):
                """Contiguous fp8 cast of tile g into a [128, 512] bf16
                container tile (byte x of partition s = fp8(enc[s, h=x]))."""
                if g >= GT or g in pk_tiles:
                    return
                src = enc_tile(g)
                pk = pk_pool.tile([128, 512], BF16, tag="pk", name=f"pk{g}")
                pk8 = pk[:].bitcast(FP8)  # [128, 1024]
                nc.gpsimd.tensor_copy(pk8[:, 0:256], src[:, 0:256])
                nc.scalar.copy(pk8[:, 256:1024], src[:, 256:1024])
                pk_tiles[g] = pk

            def tr_tile(g):
                """xbar transpose of the packed tile: [128, 4, 128] bf16.
                fp8 view: byte (2s+r) of partition j, block hb =
                enc[s, h = hb*256 + 2j + r] -> DoubleRow pairs (h, h+256)
                across block pairs at byte step 256."""
                if g >= GT or g in tr_tiles:
                    return
                trt = tr_pool.tile([128, 4, 128], BF16, tag="tr",
                                   name=f"tr{g}")
                nc.sync.dma_start(trt[:], pk_tiles[g][:], transpose=True)
                tr_tiles[g] = trt
        else:
            def cast_tile(g):
                """bf16 xbar transpose of tile g (baseline-style), issued
                here so the pipeline depth matches the packed path."""
                if g >= GT or g in tr_tiles:
                    return
                trt = tr_pool.tile([128, NHB, 128], BF16, tag="tr",
                                   name=f"tr{g}")
                nc.sync.dma_start(trt[:], enc_tile(g), transpose=True)
                tr_tiles[g] = trt

            def tr_tile(g):
                """fp8 cast of the transposed tile (plain DoubleRow
                layout, pair step 128 bytes)."""
                if g >= GT or g in trf_tiles:
                    return
                trf = pk_pool.tile([128, NHB, 128], FP8, tag="pk",
                                   name=f"trf{g}")
                nc.vector.tensor_copy(trf[:, 0:4, :], tr_tiles[g][:, 0:4, :])
                nc.scalar.copy(trf[:, 4:8, :], tr_tiles[g][:, 4:8, :])
                trf_tiles[g] = trf

        # ---------------- weight loads + transposes ----------------
        # enc prefetch first so the DMA engines start on the critical bytes
        for ghc in range(2):
            hc_dma(ghc)
        wh_nat, ws_nat = [], []
        for j in range(NHB):
            wN = wnat_pool.tile([128, H], BF16, tag="wnat")
            nc.gpsimd.dma_start(wN[:], wh_d[j * 128:(j + 1) * 128, :])
            wh_nat.append(wN)
        for ghc in range(2, PREFETCH_HC):
            hc_dma(ghc)

        # whT8 fp8 [128, 8, 1024]. PACKED: slot (2*hb+r) holds WhT rows
        # (hb*256 + 2j + r) at partition j (stride-2 sliced transposes),
        # pairing (h, h+256) to match the packed-transposed encoder.
        # Non-packed: slot hb holds WhT rows (hb*128 + j) (plain).
        whT8 = wt_pool.tile([128, NHB, H], FP8, tag="whT8")
        for jo in range(NHB):
            tp = tr_psum.tile([128, NHB * 128], BF16, tag="hp",
                              name=f"tpwh{jo}")
            if PACKED:
                wsl = wh_nat[jo][:].rearrange("o (x two) -> o two x", two=2)
                for hb in range(4):
                    for r in range(2):
                        nc.tensor.transpose(
                            tp[:, (2 * hb + r) * 128:(2 * hb + r + 1) * 128],
                            wsl[:, r, hb * 128:(hb + 1) * 128],
                            id128[:])
            else:
                for hb in range(NHB):
                    nc.tensor.transpose(tp[:, hb * 128:(hb + 1) * 128],
                                        wh_nat[jo][:, hb * 128:(hb + 1) * 128],
                                        id128[:])
            if jo % 2 == 0:
                nc.vector.tensor_copy(whT8[:, :, jo * 128:(jo + 1) * 128], tp[:])
            else:
                nc.scalar.copy(whT8[:, :, jo * 128:(jo + 1) * 128], tp[:])

        # early casts/transposes for the first tiles
        for g in range(CAST_AHEAD):
            cast_tile(g)
        for g in range(TR_AHEAD):
            tr_tile(g)

        # ---------------- query path (all batches, prologue) ----------
        for j in range(NHB):
            wN = wnat_pool.tile([128, H], BF16, tag="wnat")
            nc.gpsimd.dma_start(wN[:], ws_d[j * 128:(j + 1) * 128, :])
            ws_nat.append(wN)
        q_sb = small_pool.tile([BL, H], BF16)
        nc.gpsimd.dma_start(q_sb[:], q_d[0:BL, 0, :])
        v_row = small_pool.tile([1, H], BF16)
        nc.gpsimd.dma_start(v_row[:], v_d[0:1, :])
        v_bcast = small_pool.tile([128, H], BF16)
        nc.gpsimd.partition_broadcast(v_bcast[:], v_row[:])

        wsT = wt_pool.tile([128, NHB, H], BF16, tag="wsT")
        for j in range(NHB):
            tp = tr_psum.tile([128, NHB * 128], BF16, tag="hp")
            for hb in range(NHB):
                nc.tensor.transpose(tp[:, hb * 128:(hb + 1) * 128],
                                    ws_nat[j][:, hb * 128:(hb + 1) * 128],
                                    id128[:])
            nc.vector.tensor_copy(wsT[:, :, j * 128:(j + 1) * 128], tp[:])

        # catT[k%128, kb, b]: blocks 0..7 = ctx^T (filled at finale),
        # 8..15 = q^T
        catT = small_pool.tile([128, NKB, BL], BF16)
        qt_ps = tr_psum.tile([128, NHB * BL], BF16, tag="hp")
        for j in range(NHB):
            nc.tensor.transpose(qt_ps[:, j * BL:(j + 1) * BL],
                                q_sb[0:BL, j * 128:(j + 1) * 128], id4[:])
        nc.vector.tensor_copy(catT[:, NHB:2 * NHB, :], qt_ps[:])

        qp_ps = hp_psum.tile([BL, H], F32, tag="hp")
        for hb in range(NHB):
            for half in range(2):
                nc.tensor.matmul(qp_ps[:, half * 512:(half + 1) * 512],
                                 catT[:, NHB + hb, :],
                                 wsT[:, hb, half * 512:(half + 1) * 512],
                                 start=(hb == 0), stop=(hb == NHB - 1))
        qp_sb4 = small_pool.tile([BL, H], BF16)
        nc.scalar.copy(qp_sb4[:], qp_ps[:])
        # one DRAM bounce to move each batch's q_proj row to partition 0,
        # then broadcast across partitions (all in the prologue)
        nc.gpsimd.dma_start(qp_stash.rearrange("a (b h) -> (a b) h", b=BL),
                            qp_sb4[:])
        qpb = []
        for b in range(BL):
            qp_row = small_pool.tile([1, H], BF16, tag=f"qprow{b}")
            nc.gpsimd.dma_start(qp_row[:], qp_stash[0:1, b * H:(b + 1) * H])
            qb = small_pool.tile([128, H], BF16, tag=f"qpb{b}")
            nc.gpsimd.partition_broadcast(qb[:], qp_row[:])
            qpb.append(qb)

        ctx4_sb = small_pool.tile([BL, H], BF16)

        # ---------------- per-batch state ----------------
        scores = {}
        attnU = {}
        ctx_ps = {}
        tanh_tiles = {}

        def emit_hp(g):
            hp = hp_psum.tile([128, H], F32, tag="hp", name=f"hp{g}")
            if PACKED:
                # stationary (j, ko, s): byte (2s+r) of block (2hbp+ko) =
                # enc[s, h=(2hbp+ko)*256 + 2j + r]; pair step 256 B
                trv = tr_tiles[g][:].bitcast(FP8).rearrange(
                    "j hb (s two) -> j hb two s", two=2)  # [128,4,2,128]
                whv = whT8[:].rearrange("j (hbp ko r) o -> j hbp r ko o",
                                        hbp=2, ko=2, r=2)
                for hbp in range(2):
                    for r in range(2):
                        stat = trv[:, 2 * hbp:2 * hbp + 2, r, :]
                        for half in range(2):
                            nc.tensor.matmul(
                                hp[:, half * 512:(half + 1) * 512],
                                stat,
                                whv[:, hbp, r, :,
                                    half * 512:(half + 1) * 512],
                                start=(hbp == 0 and r == 0),
                                stop=(hbp == 1 and r == 1),
                                perf_mode=DR)
            else:
                trf = trf_tiles[g]
                for hb2 in range(4):
                    for half in range(2):
                        nc.tensor.matmul(
                            hp[:, half * 512:(half + 1) * 512],
                            trf[:, 2 * hb2:2 * hb2 + 2, :],
                            whT8[:, 2 * hb2:2 * hb2 + 2,
                                 half * 512:(half + 1) * 512],
                            start=(hb2 == 0), stop=(hb2 == 3),
                            perf_mode=DR)
            return hp

        def emit_ctx(g):
            b, st = divmod(g, NS)
            first, last = st == 0, st == NS - 1
            att = attnU[b][:, st:st + 1]
            # denominator at partition 0, ctx halves at partitions 32 / 64
            nc.tensor.matmul(ctx_ps[b][0:1, 0:1], att, ones_col[:],
                             start=first, stop=last)
            for half in range(2):
                nc.tensor.matmul(
                    ctx_ps[b][32 * (half + 1):32 * (half + 1) + 1, :],
                    att, enc_tile(g)[:, half * 512:(half + 1) * 512],
                    start=first, stop=last)

        def emit_dot(g):
            b, st = divmod(g, NS)
            junk = junk_pool.tile([128, H], BF16, tag="junk",
                                  name=f"junk{g}")
            nc.vector.scalar_tensor_tensor(
                out=junk[:], in0=tanh_tiles.pop(g)[:], scalar=1.0,
                in1=v_bcast[:],
                op0=mybir.AluOpType.mult, op1=mybir.AluOpType.mult,
                accum_out=scores[b][:, st:st + 1])

        def emit_exp_pair(g):
            b, st = divmod(g, NS)
            nc.scalar.activation(attnU[b][:, st:st + 2],
                                 scores[b][:, st:st + 2], Exp)

        def emit_denom_a(b):
            inv_d = sc_pool.tile([1, 1], F32, tag="invd", name=f"invd{b}")
            nc.vector.reciprocal(inv_d[:], ctx_ps[b][0:1, 0:1])
            inv_b = sc_pool.tile([128, 1], F32, tag="invb", name=f"invb{b}")
            nc.gpsimd.partition_broadcast(inv_b[:], inv_d[:])
            return inv_b

        def emit_ctx_store(b, inv_b):
            ctx_row = sc_pool.tile([128, 512], BF16, tag="ctxrow",
                                   name=f"ctxrow{b}")
            for half in range(2):
                p = 32 * (half + 1)
                nc.scalar.activation(ctx_row[p:p + 1, :],
                                     ctx_ps[b][p:p + 1, :], Copy,
                                     scale=inv_b[p:p + 1, 0:1])
            # tiny SB->SB partition move on the (serialized) sync queue
            nc.sync.dma_start(ctx4_sb[b:b + 1, :], ctx_row[32:65:32, :])

        inv_ds = {}
        next_ctx = [0]

        # ---------------- main loop over global tiles ----------------
        for g in range(GT):
            b, st = divmod(g, NS)
            if st == 0:
                scores[b] = sc_pool.tile([128, NS], F32, tag="scores", name=f"scores{b}")
                attnU[b] = sc_pool.tile([128, NS], BF16, tag="attnU", name=f"attnU{b}")
                ctx_ps[b] = ctx_psum.tile([128, 512], F32, tag="ctxps",
                                          name=f"ctxps{b}")
            # prefetch / pipeline feeds
            if g % 2 == 0 and g // 2 + PREFETCH_HC < GHC:
                hc_dma(g // 2 + PREFETCH_HC)
            cast_tile(g + CAST_AHEAD)
            tr_tile(g + TR_AHEAD)
            # previous batch's denominator + ctx store (off critical path)
            if b > 0:
                if st == 6:
                    inv_ds[b - 1] = emit_denom_a(b - 1)
                elif st == 7:
                    emit_ctx_store(b - 1, inv_ds.pop(b - 1))

            hp = emit_hp(g)
            # emit pending ctx matmuls; a new batch's ctx is held until
            # st==8 so the previous batch's PSUM bank has been drained
            while next_ctx[0] <= g - CTX_DELAY and next_ctx[0] < GT:
                bc = next_ctx[0] // NS
                if bc > 0 and g < bc * NS + 8:
                    break
                emit_ctx(next_ctx[0])
                next_ctx[0] += 1

            if g >= 2:
                emit_dot(g - 2)
            sum_sb = sum_pool.tile([128, H], BF16, tag="sum",
                                   name=f"sum{g}")
            nc.vector.tensor_add(sum_sb[:], hp[:], qpb[b][:])
            tanh_sb = tanh_pool.tile([128, H], BF16, tag="tanh",
                                     name=f"tanh{g}")
            nc.scalar.activation(tanh_sb[:], sum_sb[:], Tanh)
            tanh_tiles[g] = tanh_sb
            if g >= 4 and g % 2 == 0:
                emit_exp_pair(g - 4)

        # ---------------- drain the pipeline ----------------
        emit_dot(GT - 2)
        emit_dot(GT - 1)
        emit_exp_pair(GT - 4)
        emit_exp_pair(GT - 2)
        while next_ctx[0] < GT:
            emit_ctx(next_ctx[0])
            next_ctx[0] += 1
        inv_ds[BL - 1] = emit_denom_a(BL - 1)
        emit_ctx_store(BL - 1, inv_ds.pop(BL - 1))

        # ---------------- Wout prep (off critical path, xbar) ----------
        woutT = wt_pool.tile([128, NKB, H], BF16, tag="woutT")
        for j in range(NHB):
            with tc.tile_wait_until(0.45 + j * 0.03):
                woN = wonat_pool.tile([128, 2 * H], BF16, tag="wonat")
                nc.gpsimd.dma_start(woN[:], wout_d[j * 128:(j + 1) * 128, :])
            with tc.tile_wait_until(0.52 + j * 0.03):
                nc.sync.dma_start(woutT[:, :, j * 128:(j + 1) * 128], woN[:],
                                  transpose=True)

        # ---------------- finale ----------------
        # query half of the output matmul first (doesn't need ctx)
        out_ps = hp_psum.tile([BL, H], F32, tag="hp")
        with tc.tile_wait_until(0.90):
            for kb in range(NHB, NKB):
                for half in range(2):
                    nc.tensor.matmul(out_ps[:, half * 512:(half + 1) * 512],
                                     catT[:, kb, :],
                                     woutT[:, kb, half * 512:(half + 1) * 512],
                                     start=(kb == NHB), stop=False)
        ct_ps = tr_psum.tile([128, NHB * BL], BF16, tag="hp")
        for j in range(NHB):
            nc.tensor.transpose(ct_ps[:, j * BL:(j + 1) * BL],
                                ctx4_sb[0:BL, j * 128:(j + 1) * 128], id4[:])
        nc.vector.tensor_copy(catT[:, 0:NHB, :], ct_ps[:])
        for kb in range(NHB):
            for half in range(2):
                nc.tensor.matmul(out_ps[:, half * 512:(half + 1) * 512],
                                 catT[:, kb, :],
                                 woutT[:, kb, half * 512:(half + 1) * 512],
                                 start=False, stop=(kb == NHB - 1))
        out_sb = small_pool.tile([BL, H], BF16)
        nc.scalar.activation(out_sb[:], out_ps[:], Tanh)
        nc.gpsimd.dma_start(out_d[0:BL, 0, :], out_sb[:])

    nc.compile()
    return nc


_program = None


def get_program():
    global _program
    if _program is None:
        _program = _build_program()
    return _program


def run_sharded(inputs, trace=False, **kw):
    nc = get_program()
    in_maps = []
    for i in range(NCORES):
        sl = slice(i * BL, (i + 1) * BL)
        in_maps.append({
            "query": np.ascontiguousarray(inputs["query"][sl], dtype=np.float32),
            "encoder_outputs": np.ascontiguousarray(
                inputs["encoder_outputs"][sl], dtype=np.float32),
            "Ws_w": np.asarray(inputs["Ws_w"], dtype=np.float32),
            "Wh_w": np.asarray(inputs["Wh_w"], dtype=np.float32),
            "v_w": np.asarray(inputs["v_w"], dtype=np.float32),
            "Wout_w": np.asarray(inputs["Wout_w"], dtype=np.float32),
        })
    res = bass_utils.run_bass_kernel_spmd(
        nc, in_maps, core_ids=list(range(NCORES)), trace=trace, **kw)
    out = np.concatenate(
        [np.asarray(res.results[i]["out"], dtype=np.float32).reshape(BL, T, H)
         for i in range(NCORES)], axis=0)
    return out, res


def kernel(**inputs):
    out, _ = run_sharded(inputs)
    return out


# revision 21
# speedup vs baseline: 1.0511x; 1.0511x over previous
"""Bahdanau attention Trainium2 kernel (v2).

B=32, T=1, S=4096, H=1024. Data-parallel over batch across 8 NeuronCores
(4 batches/core). Per core, a single-pass streaming kernel over 128
global s-tiles (4 batches x 32 tiles), fully software-pipelined across
batch boundaries:

  - encoder streams in as half-chunks [128s, 2, 1024h] via SWDGE cast-DMA
    (fp32->bf16); tile st covers source rows [st*128, (st+1)*128) with
    partition p <-> row st*128+p
  - fp8 cast runs BEFORE the transpose: each s-tile is cast bf16->fp8
    with pair-interleaved byte layout (h and h+128 adjacent) into a
    [128, 512] bf16-container tile, so the xbar transpose moves HALF the
    bytes of the baseline (the transposed tile is consumed directly as
    DoubleRow fp8 stationary via a bitcast + stride-2 access pattern)
  - cast pieces: r=0 on vector, r=1 on scalar, 4 tiles ahead; transposes
    3 tiles ahead on the serialized sync queue
  - TensorE: h_proj fp8 DoubleRow (8 instrs/tile), inline ctx rank-1
    matmuls trailing 6 tiles (bf16, from the s-major bf16 encoder)
  - VectorE: +q_proj broadcast add
  - ScalarE: tanh; tile-pair exp with free-dim accumulation (softmax
    denominator)
  - GpSimd: the score dot (fused multiply-reduce against v), plus the
    enc-load descriptor generation and small bounces
  - q_proj broadcasts for ALL batches are built in the prologue (one
    DRAM bounce + partition_broadcast each), so batch boundaries do not
    stall; per-batch ctx rows go to SBUF via tiny sync-queue DMAs

softmax is computed without max-subtraction: |score| <= ||v||_1 ~ 26, so
exp stays comfortably inside fp32/bf16 range. Context is accumulated
unnormalized and scaled by 1/denom at batch end.

src_lengths is (faithfully to the reference) unused.
"""
import numpy as np
from contextlib import ExitStack

import concourse.bass as bass
import concourse.tile as tile
from concourse import bacc, mybir, masks
from concourse import bass_isa
from concourse import bass_utils

F32 = mybir.dt.float32
BF16 = mybir.dt.bfloat16
FP8 = mybir.dt.float8e4
Tanh = mybir.ActivationFunctionType.Tanh
Exp = mybir.ActivationFunctionType.Exp
Copy = mybir.ActivationFunctionType.Copy
DR = mybir.MatmulPerfMode.DoubleRow
DRSW = mybir.MatmulPerfMode.DoubleRowSwInterleave
# True: fp8 pack-cast BEFORE the xbar transpose (half the transpose bytes),
# consumed via DoubleRowSwInterleave. False: baseline-style bf16 transpose
# followed by an fp8 cast, consumed via plain DoubleRow.
PACKED = True

B, T, S, H = 32, 1, 4096, 1024
NCORES = 8
BL = B // NCORES       # batches per core
NS = S // 128          # s-tiles per batch
NHB = H // 128         # h blocks
NKB = 2 * H // 128     # k blocks of cat=[ctx;query]
GT = BL * NS           # global s-tiles per core
NHC = S // 256         # half-chunks (2 tiles) per batch
GHC = BL * NHC         # global half-chunks
PREFETCH_HC = 5        # half-chunks the enc DMA runs ahead
CAST_AHEAD = 8         # tiles the fp8 pack-cast runs ahead
TR_AHEAD = 6           # tiles the transpose runs ahead
CTX_DELAY = 6          # tiles the inline ctx matmuls trail


def _build_program():
    nc = bacc.Bacc("TRN2", target_bir_lowering=False, debug=False)

    q_d = nc.dram_tensor("query", (BL, T, H), F32, kind="ExternalInput").ap()
    enc_d = nc.dram_tensor("encoder_outputs", (BL, S, H), F32,
                           kind="ExternalInput").ap()
    ws_d = nc.dram_tensor("Ws_w", (H, H), F32, kind="ExternalInput").ap()
    wh_d = nc.dram_tensor("Wh_w", (H, H), F32, kind="ExternalInput").ap()
    v_d = nc.dram_tensor("v_w", (1, H), F32, kind="ExternalInput").ap()
    wout_d = nc.dram_tensor("Wout_w", (H, 2 * H), F32, kind="ExternalInput").ap()
    out_d = nc.dram_tensor("out", (BL, T, H), F32, kind="ExternalOutput").ap()
    qp_stash = nc.dram_tensor("qp_stash", (1, BL * H), BF16, kind="Internal").ap()

    with tile.TileContext(nc) as tc, ExitStack() as ctx:
        # ---------------- pools ----------------
        wt_pool = ctx.enter_context(tc.tile_pool(name="wt", bufs=1))
        wnat_pool = ctx.enter_context(tc.tile_pool(name="wnat", bufs=4))
        wonat_pool = ctx.enter_context(tc.tile_pool(name="wonat", bufs=2))
        enc_pool = ctx.enter_context(tc.tile_pool(name="encp", bufs=12))
        pk_pool = ctx.enter_context(tc.tile_pool(name="pkp", bufs=12))
        tr_pool = ctx.enter_context(tc.tile_pool(name="trp", bufs=9))
        sum_pool = ctx.enter_context(tc.tile_pool(name="sump", bufs=3))
        tanh_pool = ctx.enter_context(tc.tile_pool(name="tanhp", bufs=4))
        junk_pool = ctx.enter_context(tc.tile_pool(name="junkp", bufs=1))
        sc_pool = ctx.enter_context(tc.tile_pool(name="scp", bufs=2))
        small_pool = ctx.enter_context(tc.tile_pool(name="smallp", bufs=1))

        hp_psum = ctx.enter_context(tc.tile_pool(name="hp_ps", bufs=3, space="PSUM"))
        ctx_psum = ctx.enter_context(tc.tile_pool(name="ctx_ps", bufs=1, space="PSUM"))
        stg_psum = ctx.enter_context(tc.tile_pool(name="stg_ps", bufs=1, space="PSUM"))
        tr_psum = stg_psum  # prologue transpose staging (1 bank)

        # ---------------- constants ----------------
        id128 = small_pool.tile([128, 128], BF16)
        masks.make_identity(nc, id128[:])
        id4 = small_pool.tile([4, 4], BF16)
        masks.make_identity(nc, id4[:])
        ones_col = small_pool.tile([128, 1], BF16)
        nc.gpsimd.memset(ones_col[:], 1.0)

        # ---------------- encoder pipeline helpers ----------------
        hc_tiles = {}

        def hc_dma(ghc):
            """Load half-chunk ghc: [128, 2, 1024] bf16 (cast from fp32)."""
            if ghc in hc_tiles:
                return hc_tiles[ghc]
            b, hc = divmod(ghc, NHC)
            encN = enc_pool.tile([128, 2, H], BF16, tag="encN")
            src = enc_d[b, hc * 256:(hc + 1) * 256, :]
            nc.gpsimd.dma_start(encN[:], src.rearrange("(t p) h -> p t h", p=128))
            hc_tiles[ghc] = encN
            return encN

        def enc_tile(g):
            return hc_dma(g // 2)[:, g % 2, :]

        pk_tiles = {}
        tr_tiles = {}
        trf_tiles = {}

        if PACKED:
            def cast_tile(g):
                """Contiguous fp8 cast of tile g into a [128, 512] bf16
                container tile (byte x of partition s = fp8(enc[s, h=x]))."""
                if g >= GT or g in pk_tiles:
                    return
                src = enc_tile(g)
                pk = pk_pool.tile([128, 512], BF16, tag="pk", name=f"pk{g}")
                pk8 = pk[:].bitcast(FP8)  # [128, 1024]
                nc.scalar.copy(pk8[:], src)
                pk_tiles[g] = pk

            def tr_tile(g):
                """xbar transpose of the packed tile: [128, 4, 128] bf16.
                fp8 view: byte (2s+r) of partition j, block hb =
                enc[s, h = hb*256 + 2j + r] -> DoubleRow pairs (h, h+256)
                across block pairs at byte step 256."""
                if g >= GT or g in tr_tiles:
                    return
                trt = tr_pool.tile([128, 4, 128], BF16, tag="tr",
                                   name=f"tr{g}")
                nc.sync.dma_start(trt[:], pk_tiles[g][:], transpose=True)
                tr_tiles[g] = trt
        else:
            def cast_tile(g):
                """bf16 xbar transpose of tile g (baseline-style), issued
                here so the pipeline depth matches the packed path."""
                if g >= GT or g in tr_tiles:
                    return
                trt = tr_pool.tile([128, NHB, 128], BF16, tag="tr",
                                   name=f"tr{g}")
                nc.sync.dma_start(trt[:], enc_tile(g), transpose=True)
                tr_tiles[g] = trt

            def tr_tile(g):
                """fp8 cast of the transposed tile (plain DoubleRow
                layout, pair step 128 bytes)."""
                if g >= GT or g in trf_tiles:
                    return
                trf = pk_pool.tile([128, NHB, 128], FP8, tag="pk",
                                   name=f"trf{g}")
                nc.vector.tensor_copy(trf[:, 0:4, :], tr_tiles[g][:, 0:4, :])
                nc.scalar.copy(trf[:, 4:8, :], tr_tiles[g][:, 4:8, :])
                trf_tiles[g] = trf

        # ---------------- weight loads + transposes ----------------
        # enc prefetch first so the DMA engines start on the critical bytes
        for ghc in range(2):
            hc_dma(ghc)
        wh_nat, ws_nat = [], []
        for j in range(NHB):
            wN = wnat_pool.tile([128, H], BF16, tag="wnat")
            nc.gpsimd.dma_start(wN[:], wh_d[j * 128:(j + 1) * 128, :])
            wh_nat.append(wN)
        for ghc in range(2, PREFETCH_HC):
            hc_dma(ghc)

        # whT8 fp8 [128, 8, 1024]. PACKED: slot (2*hb+r) holds WhT rows
        # (hb*256 + 2j + r) at partition j (stride-2 sliced transposes),
        # pairing (h, h+256) to match the packed-transposed encoder.
        # Non-packed: slot hb holds WhT rows (hb*128 + j) (plain).
        whT8 = wt_pool.tile([128, NHB, H], FP8, tag="whT8")
        for jo in range(NHB):
            tp = tr_psum.tile([128, NHB * 128], BF16, tag="hp",
                              name=f"tpwh{jo}")
            if PACKED:
                wsl = wh_nat[jo][:].rearrange("o (x two) -> o two x", two=2)
                for hb in range(4):
                    for r in range(2):
                        nc.tensor.transpose(
                            tp[:, (2 * hb + r) * 128:(2 * hb + r + 1) * 128],
                            wsl[:, r, hb * 128:(hb + 1) * 128],
                            id128[:])
            else:
                for hb in range(NHB):
                    nc.tensor.transpose(tp[:, hb * 128:(hb + 1) * 128],
                                        wh_nat[jo][:, hb * 128:(hb + 1) * 128],
                                        id128[:])
            if jo % 2 == 0:
                nc.vector.tensor_copy(whT8[:, :, jo * 128:(jo + 1) * 128], tp[:])
            else:
                nc.scalar.copy(whT8[:, :, jo * 128:(jo + 1) * 128], tp[:])

        # early casts/transposes for the first tiles
        for g in range(CAST_AHEAD):
            cast_tile(g)
        for g in range(TR_AHEAD):
            tr_tile(g)

        # ---------------- query path (all batches, prologue) ----------
        for j in range(NHB):
            wN = wnat_pool.tile([128, H], BF16, tag="wnat")
            nc.gpsimd.dma_start(wN[:], ws_d[j * 128:(j + 1) * 128, :])
            ws_nat.append(wN)
        q_sb = small_pool.tile([BL, H], BF16)
        nc.gpsimd.dma_start(q_sb[:], q_d[0:BL, 0, :])
        v_row = small_pool.tile([1, H], BF16)
        nc.gpsimd.dma_start(v_row[:], v_d[0:1, :])
        v_bcast = small_pool.tile([128, H], BF16)
        nc.gpsimd.partition_broadcast(v_bcast[:], v_row[:])

        wsT = wt_pool.tile([128, NHB, H], BF16, tag="wsT")
        for j in range(NHB):
            tp = tr_psum.tile([128, NHB * 128], BF16, tag="hp")
            for hb in range(NHB):
                nc.tensor.transpose(tp[:, hb * 128:(hb + 1) * 128],
                                    ws_nat[j][:, hb * 128:(hb + 1) * 128],
                                    id128[:])
            nc.vector.tensor_copy(wsT[:, :, j * 128:(j + 1) * 128], tp[:])

        # catT[k%128, kb, b]: blocks 0..7 = ctx^T (filled at finale),
        # 8..15 = q^T
        catT = small_pool.tile([128, NKB, BL], BF16)
        qt_ps = tr_psum.tile([128, NHB * BL], BF16, tag="hp")
        for j in range(NHB):
            nc.tensor.transpose(qt_ps[:, j * BL:(j + 1) * BL],
                                q_sb[0:BL, j * 128:(j + 1) * 128], id4[:])
        nc.vector.tensor_copy(catT[:, NHB:2 * NHB, :], qt_ps[:])

        qp_ps = hp_psum.tile([BL, H], F32, tag="hp")
        for hb in range(NHB):
            for half in range(2):
                nc.tensor.matmul(qp_ps[:, half * 512:(half + 1) * 512],
                                 catT[:, NHB + hb, :],
                                 wsT[:, hb, half * 512:(half + 1) * 512],
                                 start=(hb == 0), stop=(hb == NHB - 1))
        qp_sb4 = small_pool.tile([BL, H], BF16)
        nc.scalar.copy(qp_sb4[:], qp_ps[:])
        # one DRAM bounce to move each batch's q_proj row to partition 0,
        # then broadcast across partitions (all in the prologue)
        nc.gpsimd.dma_start(qp_stash.rearrange("a (b h) -> (a b) h", b=BL),
                            qp_sb4[:])
        qpb = []
        for b in range(BL):
            qp_row = small_pool.tile([1, H], BF16, tag=f"qprow{b}")
            nc.gpsimd.dma_start(qp_row[:], qp_stash[0:1, b * H:(b + 1) * H])
            qb = small_pool.tile([128, H], BF16, tag=f"qpb{b}")
            nc.gpsimd.partition_broadcast(qb[:], qp_row[:])
            qpb.append(qb)

        ctx4_sb = small_pool.tile([BL, H], BF16)

        # ---------------- per-batch state ----------------
        scores = {}
        attnU = {}
        ctx_ps = {}
        tanh_tiles = {}

        def emit_hp(g):
            hp = hp_psum.tile([128, H], F32, tag="hp", name=f"hp{g}")
            if PACKED:
                # stationary (j, ko, s): byte (2s+r) of block (2hbp+ko) =
                # enc[s, h=(2hbp+ko)*256 + 2j + r]; pair step 256 B
                trv = tr_tiles[g][:].bitcast(FP8).rearrange(
                    "j hb (s two) -> j hb two s", two=2)  # [128,4,2,128]
                whv = whT8[:].rearrange("j (hbp ko r) o -> j hbp r ko o",
                                        hbp=2, ko=2, r=2)
                for hbp in range(2):
                    for r in range(2):
                        stat = trv[:, 2 * hbp:2 * hbp + 2, r, :]
                        for half in range(2):
                            nc.tensor.matmul(
                                hp[:, half * 512:(half + 1) * 512],
                                stat,
                                whv[:, hbp, r, :,
                                    half * 512:(half + 1) * 512],
                                start=(hbp == 0 and r == 0),
                                stop=(hbp == 1 and r == 1),
                                perf_mode=DR)
            else:
                trf = trf_tiles[g]
                for hb2 in range(4):
                    for half in range(2):
                        nc.tensor.matmul(
                            hp[:, half * 512:(half + 1) * 512],
                            trf[:, 2 * hb2:2 * hb2 + 2, :],
                            whT8[:, 2 * hb2:2 * hb2 + 2,
                                 half * 512:(half + 1) * 512],
                            start=(hb2 == 0), stop=(hb2 == 3),
                            perf_mode=DR)
            return hp

        def emit_ctx(g):
            b, st = divmod(g, NS)
            first, last = st == 0, st == NS - 1
            att = attnU[b][:, st:st + 1]
            # denominator at partition 0, ctx halves at partitions 32 / 64
            nc.tensor.matmul(ctx_ps[b][0:1, 0:1], att, ones_col[:],
                             start=first, stop=last)
            for half in range(2):
                nc.tensor.matmul(
                    ctx_ps[b][32 * (half + 1):32 * (half + 1) + 1, :],
                    att, enc_tile(g)[:, half * 512:(half + 1) * 512],
                    start=first, stop=last)

        def emit_dot(g):
            b, st = divmod(g, NS)
            junk = junk_pool.tile([128, H], BF16, tag="junk",
                                  name=f"junk{g}")
            nc.vector.scalar_tensor_tensor(
                out=junk[:], in0=tanh_tiles.pop(g)[:], scalar=1.0,
                in1=v_bcast[:],
                op0=mybir.AluOpType.mult, op1=mybir.AluOpType.mult,
                accum_out=scores[b][:, st:st + 1])

        def emit_exp_pair(g):
            b, st = divmod(g, NS)
            nc.scalar.activation(attnU[b][:, st:st + 2],
                                 scores[b][:, st:st + 2], Exp)

        def emit_denom_a(b):
            inv_d = sc_pool.tile([1, 1], F32, tag="invd", name=f"invd{b}")
            nc.vector.reciprocal(inv_d[:], ctx_ps[b][0:1, 0:1])
            inv_b = sc_pool.tile([128, 1], F32, tag="invb", name=f"invb{b}")
            nc.gpsimd.partition_broadcast(inv_b[:], inv_d[:])
            return inv_b

        def emit_ctx_store(b, inv_b):
            ctx_row = sc_pool.tile([128, 512], BF16, tag="ctxrow",
                                   name=f"ctxrow{b}")
            for half in range(2):
                p = 32 * (half + 1)
                nc.scalar.activation(ctx_row[p:p + 1, :],
                                     ctx_ps[b][p:p + 1, :], Copy,
                                     scale=inv_b[p:p + 1, 0:1])
            # tiny SB->SB partition move on the (serialized) sync queue
            nc.sync.dma_start(ctx4_sb[b:b + 1, :], ctx_row[32:65:32, :])

        inv_ds = {}
        next_ctx = [0]

        # ---------------- main loop over global tiles ----------------
        for g in range(GT):
            b, st = divmod(g, NS)
            if st == 0:
                scores[b] = sc_pool.tile([128, NS], F32, tag="scores", name=f"scores{b}")
                attnU[b] = sc_pool.tile([128, NS], BF16, tag="attnU", name=f"attnU{b}")
                ctx_ps[b] = ctx_psum.tile([128, 512], F32, tag="ctxps",
                                          name=f"ctxps{b}")
            # prefetch / pipeline feeds
            if g % 2 == 0 and g // 2 + PREFETCH_HC < GHC:
                hc_dma(g // 2 + PREFETCH_HC)
            cast_tile(g + CAST_AHEAD)
            tr_tile(g + TR_AHEAD)
            # previous batch's denominator + ctx store (off critical path)
            if b > 0:
                if st == 6:
                    inv_ds[b - 1] = emit_denom_a(b - 1)
                elif st == 7:
                    emit_ctx_store(b - 1, inv_ds.pop(b - 1))

            hp = emit_hp(g)
            # emit pending ctx matmuls; a new batch's ctx is held until
            # st==8 so the previous batch's PSUM bank has been drained
            while next_ctx[0] <= g - CTX_DELAY and next_ctx[0] < GT:
                bc = next_ctx[0] // NS
                if bc > 0 and g < bc * NS + 8:
                    break
                emit_ctx(next_ctx[0])
                next_ctx[0] += 1

            if g >= 2:
                emit_dot(g - 2)
            sum_sb = sum_pool.tile([128, H], BF16, tag="sum",
                                   name=f"sum{g}")
            nc.vector.tensor_add(sum_sb[:], hp[:], qpb[b][:])
            tanh_sb = tanh_pool.tile([128, H], BF16, tag="tanh",
                                     name=f"tanh{g}")
            nc.scalar.activation(tanh_sb[:], sum_sb[:], Tanh)
            tanh_tiles[g] = tanh_sb
            if g >= 4 and g % 2 == 0:
                emit_exp_pair(g - 4)

        # ---------------- drain the pipeline ----------------
        emit_dot(GT - 2)
        emit_dot(GT - 1)
        emit_exp_pair(GT - 4)
        emit_exp_pair(GT - 2)
        while next_ctx[0] < GT:
            emit_ctx(next_ctx[0])
            next_ctx[0] += 1
        inv_ds[BL - 1] = emit_denom_a(BL - 1)
        emit_ctx_store(BL - 1, inv_ds.pop(BL - 1))

        # ---------------- Wout prep (off critical path, xbar) ----------
        woutT = wt_pool.tile([128, NKB, H], BF16, tag="woutT")
        for j in range(NHB):
            with tc.tile_wait_until(0.45 + j * 0.03):
                woN = wonat_pool.tile([128, 2 * H], BF16, tag="wonat")
                nc.gpsimd.dma_start(woN[:], wout_d[j * 128:(j + 1) * 128, :])
            with tc.tile_wait_until(0.52 + j * 0.03):
                nc.sync.dma_start(woutT[:, :, j * 128:(j + 1) * 128], woN[:],
                                  transpose=True)

        # ---------------- finale ----------------
        # query half of the output matmul first (doesn't need ctx)
        out_ps = hp_psum.tile([BL, H], F32, tag="hp")
        with tc.tile_wait_until(0.90):
            for kb in range(NHB, NKB):
                for half in range(2):
                    nc.tensor.matmul(out_ps[:, half * 512:(half + 1) * 512],
                                     catT[:, kb, :],
                                     woutT[:, kb, half * 512:(half + 1) * 512],
                                     start=(kb == NHB), stop=False)
        ct_ps = tr_psum.tile([128, NHB * BL], BF16, tag="hp")
        for j in range(NHB):
            nc.tensor.transpose(ct_ps[:, j * BL:(j + 1) * BL],
                                ctx4_sb[0:BL, j * 128:(j + 1) * 128], id4[:])
        nc.vector.tensor_copy(catT[:, 0:NHB, :], ct_ps[:])
        for kb in range(NHB):
            for half in range(2):
                nc.tensor.matmul(out_ps[:, half * 512:(half + 1) * 512],
                                 catT[:, kb, :],
                                 woutT[:, kb, half * 512:(half + 1) * 512],
                                 start=False, stop=(kb == NHB - 1))
        out_sb = small_pool.tile([BL, H], BF16)
        nc.scalar.activation(out_sb[:], out_ps[:], Tanh)
        nc.gpsimd.dma_start(out_d[0:BL, 0, :], out_sb[:])

    nc.compile()
    return nc


_program = None


def get_program():
    global _program
    if _program is None:
        _program = _build_program()
    return _program


def run_sharded(inputs, trace=False, **kw):
    nc = get_program()
    in_maps = []
    for i in range(NCORES):
        sl = slice(i * BL, (i + 1) * BL)
        in_maps.append({
            "query": np.ascontiguousarray(inputs["query"][sl], dtype=np.float32),
            "encoder_outputs": np.ascontiguousarray(
                inputs["encoder_outputs"][sl], dtype=np.float32),
            "Ws_w": np.asarray(inputs["Ws_w"], dtype=np.float32),
            "Wh_w": np.asarray(inputs["Wh_w"], dtype=np.float32),
            "v_w": np.asarray(inputs["v_w"], dtype=np.float32),
            "Wout_w": np.asarray(inputs["Wout_w"], dtype=np.float32),
        })
    res = bass_utils.run_bass_kernel_spmd(
        nc, in_maps, core_ids=list(range(NCORES)), trace=trace, **kw)
    out = np.concatenate(
        [np.asarray(res.results[i]["out"], dtype=np.float32).reshape(BL, T, H)
         for i in range(NCORES)], axis=0)
    return out, res


def kernel(**inputs):
    out, _ = run_sharded(inputs)
    return out


# revision 22
# speedup vs baseline: 1.0892x; 1.0363x over previous
"""Bahdanau attention Trainium2 kernel (v2).

B=32, T=1, S=4096, H=1024. Data-parallel over batch across 8 NeuronCores
(4 batches/core). Per core, a single-pass streaming kernel over 128
global s-tiles (4 batches x 32 tiles), fully software-pipelined across
batch boundaries:

  - encoder streams in as half-chunks [128s, 2, 1024h] via SWDGE cast-DMA
    (fp32->bf16); tile st covers source rows [st*128, (st+1)*128) with
    partition p <-> row st*128+p
  - fp8 cast runs BEFORE the transpose: each s-tile is cast bf16->fp8
    with pair-interleaved byte layout (h and h+128 adjacent) into a
    [128, 512] bf16-container tile, so the xbar transpose moves HALF the
    bytes of the baseline (the transposed tile is consumed directly as
    DoubleRow fp8 stationary via a bitcast + stride-2 access pattern)
  - cast pieces: r=0 on vector, r=1 on scalar, 4 tiles ahead; transposes
    3 tiles ahead on the serialized sync queue
  - TensorE: h_proj fp8 DoubleRow (8 instrs/tile), inline ctx rank-1
    matmuls trailing 6 tiles (bf16, from the s-major bf16 encoder)
  - VectorE: +q_proj broadcast add
  - ScalarE: tanh; tile-pair exp with free-dim accumulation (softmax
    denominator)
  - GpSimd: the score dot (fused multiply-reduce against v), plus the
    enc-load descriptor generation and small bounces
  - q_proj broadcasts for ALL batches are built in the prologue (one
    DRAM bounce + partition_broadcast each), so batch boundaries do not
    stall; per-batch ctx rows go to SBUF via tiny sync-queue DMAs

softmax is computed without max-subtraction: |score| <= ||v||_1 ~ 26, so
exp stays comfortably inside fp32/bf16 range. Context is accumulated
unnormalized and scaled by 1/denom at batch end.

src_lengths is (faithfully to the reference) unused.
"""
import numpy as np
from contextlib import ExitStack

import concourse.bass as bass
import concourse.tile as tile
from concourse import bacc, mybir, masks
from concourse import bass_isa
from concourse import bass_utils

F32 = mybir.dt.float32
BF16 = mybir.dt.bfloat16
FP8 = mybir.dt.float8e4
Tanh = mybir.ActivationFunctionType.Tanh
Exp = mybir.ActivationFunctionType.Exp
Copy = mybir.ActivationFunctionType.Copy
DR = mybir.MatmulPerfMode.DoubleRow
DRSW = mybir.MatmulPerfMode.DoubleRowSwInterleave
# True: fp8 pack-cast BEFORE the xbar transpose (half the transpose bytes),
# consumed via DoubleRowSwInterleave. False: baseline-style bf16 transpose
# followed by an fp8 cast, consumed via plain DoubleRow.
PACKED = True

B, T, S, H = 32, 1, 4096, 1024
NCORES = 8
BL = B // NCORES       # batches per core
NS = S // 128          # s-tiles per batch
NHB = H // 128         # h blocks
NKB = 2 * H // 128     # k blocks of cat=[ctx;query]
GT = BL * NS           # global s-tiles per core
NHC = S // 256         # half-chunks (2 tiles) per batch
GHC = BL * NHC         # global half-chunks
PREFETCH_HC = 8        # half-chunks the enc DMA runs ahead
CAST_AHEAD = 8         # tiles the fp8 pack-cast runs ahead
TR_AHEAD = 6           # tiles the transpose runs ahead
CTX_DELAY = 6          # tiles the inline ctx matmuls trail


def _build_program():
    nc = bacc.Bacc("TRN2", target_bir_lowering=False, debug=False)

    q_d = nc.dram_tensor("query", (BL, T, H), F32, kind="ExternalInput").ap()
    enc_d = nc.dram_tensor("encoder_outputs", (BL, S, H), F32,
                           kind="ExternalInput").ap()
    ws_d = nc.dram_tensor("Ws_w", (H, H), F32, kind="ExternalInput").ap()
    wh_d = nc.dram_tensor("Wh_w", (H, H), F32, kind="ExternalInput").ap()
    v_d = nc.dram_tensor("v_w", (1, H), F32, kind="ExternalInput").ap()
    wout_d = nc.dram_tensor("Wout_w", (H, 2 * H), F32, kind="ExternalInput").ap()
    out_d = nc.dram_tensor("out", (BL, T, H), F32, kind="ExternalOutput").ap()
    qp_stash = nc.dram_tensor("qp_stash", (1, BL * H), BF16, kind="Internal").ap()

    with tile.TileContext(nc) as tc, ExitStack() as ctx:
        # ---------------- pools ----------------
        wt_pool = ctx.enter_context(tc.tile_pool(name="wt", bufs=1))
        wnat_pool = ctx.enter_context(tc.tile_pool(name="wnat", bufs=4))
        wonat_pool = ctx.enter_context(tc.tile_pool(name="wonat", bufs=2))
        enc_pool = ctx.enter_context(tc.tile_pool(name="encp", bufs=12))
        pk_pool = ctx.enter_context(tc.tile_pool(name="pkp", bufs=12))
        tr_pool = ctx.enter_context(tc.tile_pool(name="trp", bufs=9))
        sum_pool = ctx.enter_context(tc.tile_pool(name="sump", bufs=3))
        tanh_pool = ctx.enter_context(tc.tile_pool(name="tanhp", bufs=4))
        junk_pool = ctx.enter_context(tc.tile_pool(name="junkp", bufs=1))
        sc_pool = ctx.enter_context(tc.tile_pool(name="scp", bufs=2))
        small_pool = ctx.enter_context(tc.tile_pool(name="smallp", bufs=1))

        hp_psum = ctx.enter_context(tc.tile_pool(name="hp_ps", bufs=3, space="PSUM"))
        ctx_psum = ctx.enter_context(tc.tile_pool(name="ctx_ps", bufs=1, space="PSUM"))
        stg_psum = ctx.enter_context(tc.tile_pool(name="stg_ps", bufs=1, space="PSUM"))
        tr_psum = stg_psum  # prologue transpose staging (1 bank)

        # ---------------- constants ----------------
        id128 = small_pool.tile([128, 128], BF16)
        masks.make_identity(nc, id128[:])
        id4 = small_pool.tile([4, 4], BF16)
        masks.make_identity(nc, id4[:])
        ones_col = small_pool.tile([128, 1], BF16)
        nc.gpsimd.memset(ones_col[:], 1.0)

        # ---------------- encoder pipeline helpers ----------------
        hc_tiles = {}

        def hc_dma(ghc):
            """Load half-chunk ghc: [128, 2, 1024] bf16 (cast from fp32)."""
            if ghc in hc_tiles:
                return hc_tiles[ghc]
            b, hc = divmod(ghc, NHC)
            encN = enc_pool.tile([128, 2, H], BF16, tag="encN")
            src = enc_d[b, hc * 256:(hc + 1) * 256, :]
            nc.gpsimd.dma_start(encN[:], src.rearrange("(t p) h -> p t h", p=128))
            hc_tiles[ghc] = encN
            return encN

        def enc_tile(g):
            return hc_dma(g // 2)[:, g % 2, :]

        pk_tiles = {}
        tr_tiles = {}
        trf_tiles = {}

        if PACKED:
            def cast_tile(g):
                """Contiguous fp8 cast of tile g into a [128, 512] bf16
                container tile (byte x of partition s = fp8(enc[s, h=x]))."""
                if g >= GT or g in pk_tiles:
                    return
                src = enc_tile(g)
                pk = pk_pool.tile([128, 512], BF16, tag="pk", name=f"pk{g}")
                pk8 = pk[:].bitcast(FP8)  # [128, 1024]
                nc.scalar.copy(pk8[:], src)
                pk_tiles[g] = pk

            def tr_tile(g):
                """xbar transpose of the packed tile: [128, 4, 128] bf16.
                fp8 view: byte (2s+r) of partition j, block hb =
                enc[s, h = hb*256 + 2j + r] -> DoubleRow pairs (h, h+256)
                across block pairs at byte step 256."""
                if g >= GT or g in tr_tiles:
                    return
                trt = tr_pool.tile([128, 4, 128], BF16, tag="tr",
                                   name=f"tr{g}")
                nc.sync.dma_start(trt[:], pk_tiles[g][:], transpose=True)
                tr_tiles[g] = trt
        else:
            def cast_tile(g):
                """bf16 xbar transpose of tile g (baseline-style), issued
                here so the pipeline depth matches the packed path."""
                if g >= GT or g in tr_tiles:
                    return
                trt = tr_pool.tile([128, NHB, 128], BF16, tag="tr",
                                   name=f"tr{g}")
                nc.sync.dma_start(trt[:], enc_tile(g), transpose=True)
                tr_tiles[g] = trt

            def tr_tile(g):
                """fp8 cast of the transposed tile (plain DoubleRow
                layout, pair step 128 bytes)."""
                if g >= GT or g in trf_tiles:
                    return
                trf = pk_pool.tile([128, NHB, 128], FP8, tag="pk",
                                   name=f"trf{g}")
                nc.vector.tensor_copy(trf[:, 0:4, :], tr_tiles[g][:, 0:4, :])
                nc.scalar.copy(trf[:, 4:8, :], tr_tiles[g][:, 4:8, :])
                trf_tiles[g] = trf

        # ---------------- weight loads + transposes ----------------
        # enc prefetch first so the DMA engines start on the critical bytes
        for ghc in range(2):
            hc_dma(ghc)
        wh_nat, ws_nat = [], []
        for j in range(NHB):
            wN = wnat_pool.tile([128, H], BF16, tag="wnat")
            nc.gpsimd.dma_start(wN[:], wh_d[j * 128:(j + 1) * 128, :])
            wh_nat.append(wN)
        for ghc in range(2, PREFETCH_HC):
            hc_dma(ghc)

        # whT8 fp8 [128, 8, 1024]. PACKED: slot (2*hb+r) holds WhT rows
        # (hb*256 + 2j + r) at partition j (stride-2 sliced transposes),
        # pairing (h, h+256) to match the packed-transposed encoder.
        # Non-packed: slot hb holds WhT rows (hb*128 + j) (plain).
        whT8 = wt_pool.tile([128, NHB, H], FP8, tag="whT8")
        for jo in range(NHB):
            tp = tr_psum.tile([128, NHB * 128], BF16, tag="hp",
                              name=f"tpwh{jo}")
            if PACKED:
                wsl = wh_nat[jo][:].rearrange("o (x two) -> o two x", two=2)
                for hb in range(4):
                    for r in range(2):
                        nc.tensor.transpose(
                            tp[:, (2 * hb + r) * 128:(2 * hb + r + 1) * 128],
                            wsl[:, r, hb * 128:(hb + 1) * 128],
                            id128[:])
            else:
                for hb in range(NHB):
                    nc.tensor.transpose(tp[:, hb * 128:(hb + 1) * 128],
                                        wh_nat[jo][:, hb * 128:(hb + 1) * 128],
                                        id128[:])
            if jo % 2 == 0:
                nc.vector.tensor_copy(whT8[:, :, jo * 128:(jo + 1) * 128], tp[:])
            else:
                nc.scalar.copy(whT8[:, :, jo * 128:(jo + 1) * 128], tp[:])

        # early casts/transposes for the first tiles
        for g in range(CAST_AHEAD):
            cast_tile(g)
        for g in range(TR_AHEAD):
            tr_tile(g)

        # ---------------- query path (all batches, prologue) ----------
        for j in range(NHB):
            wN = wnat_pool.tile([128, H], BF16, tag="wnat")
            nc.gpsimd.dma_start(wN[:], ws_d[j * 128:(j + 1) * 128, :])
            ws_nat.append(wN)
        q_sb = small_pool.tile([BL, H], BF16)
        nc.gpsimd.dma_start(q_sb[:], q_d[0:BL, 0, :])
        v_row = small_pool.tile([1, H], BF16)
        nc.gpsimd.dma_start(v_row[:], v_d[0:1, :])
        v_bcast = small_pool.tile([128, H], BF16)
        nc.gpsimd.partition_broadcast(v_bcast[:], v_row[:])

        wsT = wt_pool.tile([128, NHB, H], BF16, tag="wsT")
        for j in range(NHB):
            tp = tr_psum.tile([128, NHB * 128], BF16, tag="hp")
            for hb in range(NHB):
                nc.tensor.transpose(tp[:, hb * 128:(hb + 1) * 128],
                                    ws_nat[j][:, hb * 128:(hb + 1) * 128],
                                    id128[:])
            nc.vector.tensor_copy(wsT[:, :, j * 128:(j + 1) * 128], tp[:])

        # catT[k%128, kb, b]: blocks 0..7 = ctx^T (filled at finale),
        # 8..15 = q^T
        catT = small_pool.tile([128, NKB, BL], BF16)
        qt_ps = tr_psum.tile([128, NHB * BL], BF16, tag="hp")
        for j in range(NHB):
            nc.tensor.transpose(qt_ps[:, j * BL:(j + 1) * BL],
                                q_sb[0:BL, j * 128:(j + 1) * 128], id4[:])
        nc.vector.tensor_copy(catT[:, NHB:2 * NHB, :], qt_ps[:])

        qp_ps = hp_psum.tile([BL, H], F32, tag="hp")
        for hb in range(NHB):
            for half in range(2):
                nc.tensor.matmul(qp_ps[:, half * 512:(half + 1) * 512],
                                 catT[:, NHB + hb, :],
                                 wsT[:, hb, half * 512:(half + 1) * 512],
                                 start=(hb == 0), stop=(hb == NHB - 1))
        qp_sb4 = small_pool.tile([BL, H], BF16)
        nc.scalar.copy(qp_sb4[:], qp_ps[:])
        # one DRAM bounce to move each batch's q_proj row to partition 0,
        # then broadcast across partitions (all in the prologue)
        nc.gpsimd.dma_start(qp_stash.rearrange("a (b h) -> (a b) h", b=BL),
                            qp_sb4[:])
        qpb = []
        for b in range(BL):
            qp_row = small_pool.tile([1, H], BF16, tag=f"qprow{b}")
            nc.gpsimd.dma_start(qp_row[:], qp_stash[0:1, b * H:(b + 1) * H])
            qb = small_pool.tile([128, H], BF16, tag=f"qpb{b}")
            nc.gpsimd.partition_broadcast(qb[:], qp_row[:])
            qpb.append(qb)

        ctx4_sb = small_pool.tile([BL, H], BF16)

        # ---------------- per-batch state ----------------
        scores = {}
        attnU = {}
        ctx_ps = {}
        tanh_tiles = {}

        def emit_hp(g):
            hp = hp_psum.tile([128, H], F32, tag="hp", name=f"hp{g}")
            if PACKED:
                # stationary (j, ko, s): byte (2s+r) of block (2hbp+ko) =
                # enc[s, h=(2hbp+ko)*256 + 2j + r]; pair step 256 B
                trv = tr_tiles[g][:].bitcast(FP8).rearrange(
                    "j hb (s two) -> j hb two s", two=2)  # [128,4,2,128]
                whv = whT8[:].rearrange("j (hbp ko r) o -> j hbp r ko o",
                                        hbp=2, ko=2, r=2)
                for hbp in range(2):
                    for r in range(2):
                        stat = trv[:, 2 * hbp:2 * hbp + 2, r, :]
                        for half in range(2):
                            nc.tensor.matmul(
                                hp[:, half * 512:(half + 1) * 512],
                                stat,
                                whv[:, hbp, r, :,
                                    half * 512:(half + 1) * 512],
                                start=(hbp == 0 and r == 0),
                                stop=(hbp == 1 and r == 1),
                                perf_mode=DR)
            else:
                trf = trf_tiles[g]
                for hb2 in range(4):
                    for half in range(2):
                        nc.tensor.matmul(
                            hp[:, half * 512:(half + 1) * 512],
                            trf[:, 2 * hb2:2 * hb2 + 2, :],
                            whT8[:, 2 * hb2:2 * hb2 + 2,
                                 half * 512:(half + 1) * 512],
                            start=(hb2 == 0), stop=(hb2 == 3),
                            perf_mode=DR)
            return hp

        def emit_ctx(g):
            b, st = divmod(g, NS)
            first, last = st == 0, st == NS - 1
            att = attnU[b][:, st:st + 1]
            # denominator at partition 0, ctx halves at partitions 32 / 64
            nc.tensor.matmul(ctx_ps[b][0:1, 0:1], att, ones_col[:],
                             start=first, stop=last)
            for half in range(2):
                nc.tensor.matmul(
                    ctx_ps[b][32 * (half + 1):32 * (half + 1) + 1, :],
                    att, enc_tile(g)[:, half * 512:(half + 1) * 512],
                    start=first, stop=last)

        def emit_dot(g):
            b, st = divmod(g, NS)
            junk = junk_pool.tile([128, H], BF16, tag="junk",
                                  name=f"junk{g}")
            nc.vector.scalar_tensor_tensor(
                out=junk[:], in0=tanh_tiles.pop(g)[:], scalar=1.0,
                in1=v_bcast[:],
                op0=mybir.AluOpType.mult, op1=mybir.AluOpType.mult,
                accum_out=scores[b][:, st:st + 1])

        def emit_exp_pair(g):
            b, st = divmod(g, NS)
            nc.scalar.activation(attnU[b][:, st:st + 2],
                                 scores[b][:, st:st + 2], Exp)

        def emit_denom_a(b):
            inv_d = sc_pool.tile([1, 1], F32, tag="invd", name=f"invd{b}")
            nc.vector.reciprocal(inv_d[:], ctx_ps[b][0:1, 0:1])
            inv_b = sc_pool.tile([128, 1], F32, tag="invb", name=f"invb{b}")
            nc.gpsimd.partition_broadcast(inv_b[:], inv_d[:])
            return inv_b

        def emit_ctx_store(b, inv_b):
            ctx_row = sc_pool.tile([128, 512], BF16, tag="ctxrow",
                                   name=f"ctxrow{b}")
            for half in range(2):
                p = 32 * (half + 1)
                nc.scalar.activation(ctx_row[p:p + 1, :],
                                     ctx_ps[b][p:p + 1, :], Copy,
                                     scale=inv_b[p:p + 1, 0:1])
            # tiny SB->SB partition move on the (serialized) sync queue
            nc.sync.dma_start(ctx4_sb[b:b + 1, :], ctx_row[32:65:32, :])

        inv_ds = {}
        next_ctx = [0]

        # ---------------- main loop over global tiles ----------------
        for g in range(GT):
            b, st = divmod(g, NS)
            if st == 0:
                scores[b] = sc_pool.tile([128, NS], F32, tag="scores", name=f"scores{b}")
                attnU[b] = sc_pool.tile([128, NS], BF16, tag="attnU", name=f"attnU{b}")
                ctx_ps[b] = ctx_psum.tile([128, 512], F32, tag="ctxps",
                                          name=f"ctxps{b}")
            # prefetch / pipeline feeds
            if g % 2 == 0 and g // 2 + PREFETCH_HC < GHC:
                hc_dma(g // 2 + PREFETCH_HC)
            cast_tile(g + CAST_AHEAD)
            tr_tile(g + TR_AHEAD)
            # previous batch's denominator + ctx store (off critical path)
            if b > 0:
                if st == 6:
                    inv_ds[b - 1] = emit_denom_a(b - 1)
                elif st == 7:
                    emit_ctx_store(b - 1, inv_ds.pop(b - 1))

            hp = emit_hp(g)
            # emit pending ctx matmuls; a new batch's ctx is held until
            # st==8 so the previous batch's PSUM bank has been drained
            while next_ctx[0] <= g - CTX_DELAY and next_ctx[0] < GT:
                bc = next_ctx[0] // NS
                if bc > 0 and g < bc * NS + 8:
                    break
                emit_ctx(next_ctx[0])
                next_ctx[0] += 1

            if g >= 2:
                emit_dot(g - 2)
            sum_sb = sum_pool.tile([128, H], BF16, tag="sum",
                                   name=f"sum{g}")
            nc.vector.tensor_add(sum_sb[:], hp[:], qpb[b][:])
            tanh_sb = tanh_pool.tile([128, H], BF16, tag="tanh",
                                     name=f"tanh{g}")
            nc.scalar.activation(tanh_sb[:], sum_sb[:], Tanh)
            tanh_tiles[g] = tanh_sb
            if g >= 4 and g % 2 == 0:
                emit_exp_pair(g - 4)

        # ---------------- drain the pipeline ----------------
        emit_dot(GT - 2)
        emit_dot(GT - 1)
        emit_exp_pair(GT - 4)
        emit_exp_pair(GT - 2)
        while next_ctx[0] < GT:
            emit_ctx(next_ctx[0])
            next_ctx[0] += 1
        inv_ds[BL - 1] = emit_denom_a(BL - 1)
        emit_ctx_store(BL - 1, inv_ds.pop(BL - 1))

        # ---------------- Wout prep (off critical path, xbar) ----------
        woutT = wt_pool.tile([128, NKB, H], BF16, tag="woutT")
        for j in range(NHB):
            with tc.tile_wait_until(0.45 + j * 0.03):
                woN = wonat_pool.tile([128, 2 * H], BF16, tag="wonat")
                nc.gpsimd.dma_start(woN[:], wout_d[j * 128:(j + 1) * 128, :])
            with tc.tile_wait_until(0.52 + j * 0.03):
                nc.sync.dma_start(woutT[:, :, j * 128:(j + 1) * 128], woN[:],
                                  transpose=True)

        # ---------------- finale ----------------
        # query half of the output matmul first (doesn't need ctx)
        out_ps = hp_psum.tile([BL, H], F32, tag="hp")
        with tc.tile_wait_until(0.90):
            for kb in range(NHB, NKB):
                for half in range(2):
                    nc.tensor.matmul(out_ps[:, half * 512:(half + 1) * 512],
                                     catT[:, kb, :],
                                     woutT[:, kb, half * 512:(half + 1) * 512],
                                     start=(kb == NHB), stop=False)
        ct_ps = tr_psum.tile([128, NHB * BL], BF16, tag="hp")
        for j in range(NHB):
            nc.tensor.transpose(ct_ps[:, j * BL:(j + 1) * BL],
                                ctx4_sb[0:BL, j * 128:(j + 1) * 128], id4[:])
        nc.vector.tensor_copy(catT[:, 0:NHB, :], ct_ps[:])
        for kb in range(NHB):
            for half in range(2):
                nc.tensor.matmul(out_ps[:, half * 512:(half + 1) * 512],
                                 catT[:, kb, :],
                                 woutT[:, kb, half * 512:(half + 1) * 512],
                                 start=False, stop=(kb == NHB - 1))
        out_sb = small_pool.tile([BL, H], BF16)
        nc.scalar.activation(out_sb[:], out_ps[:], Tanh)
        nc.gpsimd.dma_start(out_d[0:BL, 0, :], out_sb[:])

    nc.compile()
    return nc


_program = None


def get_program():
    global _program
    if _program is None:
        _program = _build_program()
    return _program


def run_sharded(inputs, trace=False, **kw):
    nc = get_program()
    in_maps = []
    for i in range(NCORES):
        sl = slice(i * BL, (i + 1) * BL)
        in_maps.append({
            "query": np.ascontiguousarray(inputs["query"][sl], dtype=np.float32),
            "encoder_outputs": np.ascontiguousarray(
                inputs["encoder_outputs"][sl], dtype=np.float32),
            "Ws_w": np.asarray(inputs["Ws_w"], dtype=np.float32),
            "Wh_w": np.asarray(inputs["Wh_w"], dtype=np.float32),
            "v_w": np.asarray(inputs["v_w"], dtype=np.float32),
            "Wout_w": np.asarray(inputs["Wout_w"], dtype=np.float32),
        })
    res = bass_utils.run_bass_kernel_spmd(
        nc, in_maps, core_ids=list(range(NCORES)), trace=trace, **kw)
    out = np.concatenate(
        [np.asarray(res.results[i]["out"], dtype=np.float32).reshape(BL, T, H)
         for i in range(NCORES)], axis=0)
    return out, res


def kernel(**inputs):
    out, _ = run_sharded(inputs)
    return out


# revision 24
# speedup vs baseline: 1.1261x; 1.0338x over previous
"""Bahdanau attention Trainium2 kernel (v2).

B=32, T=1, S=4096, H=1024. Data-parallel over batch across 8 NeuronCores
(4 batches/core). Per core, a single-pass streaming kernel over 128
global s-tiles (4 batches x 32 tiles), fully software-pipelined across
batch boundaries:

  - encoder streams in as half-chunks [128s, 2, 1024h] via SWDGE cast-DMA
    (fp32->bf16); tile st covers source rows [st*128, (st+1)*128) with
    partition p <-> row st*128+p
  - fp8 cast runs BEFORE the transpose: each s-tile is cast bf16->fp8
    with pair-interleaved byte layout (h and h+128 adjacent) into a
    [128, 512] bf16-container tile, so the xbar transpose moves HALF the
    bytes of the baseline (the transposed tile is consumed directly as
    DoubleRow fp8 stationary via a bitcast + stride-2 access pattern)
  - cast pieces: r=0 on vector, r=1 on scalar, 4 tiles ahead; transposes
    3 tiles ahead on the serialized sync queue
  - TensorE: h_proj fp8 DoubleRow (8 instrs/tile), inline ctx rank-1
    matmuls trailing 6 tiles (bf16, from the s-major bf16 encoder)
  - VectorE: +q_proj broadcast add
  - ScalarE: tanh; tile-pair exp with free-dim accumulation (softmax
    denominator)
  - GpSimd: the score dot (fused multiply-reduce against v), plus the
    enc-load descriptor generation and small bounces
  - q_proj broadcasts for ALL batches are built in the prologue (one
    DRAM bounce + partition_broadcast each), so batch boundaries do not
    stall; per-batch ctx rows go to SBUF via tiny sync-queue DMAs

softmax is computed without max-subtraction: |score| <= ||v||_1 ~ 26, so
exp stays comfortably inside fp32/bf16 range. Context is accumulated
unnormalized and scaled by 1/denom at batch end.

src_lengths is (faithfully to the reference) unused.
"""
import numpy as np
from contextlib import ExitStack

import concourse.bass as bass
import concourse.tile as tile
from concourse import bacc, mybir, masks
from concourse import bass_isa
from concourse import bass_utils

F32 = mybir.dt.float32
BF16 = mybir.dt.bfloat16
FP8 = mybir.dt.float8e4
Tanh = mybir.ActivationFunctionType.Tanh
Exp = mybir.ActivationFunctionType.Exp
Copy = mybir.ActivationFunctionType.Copy
DR = mybir.MatmulPerfMode.DoubleRow
DRSW = mybir.MatmulPerfMode.DoubleRowSwInterleave
# True: fp8 pack-cast BEFORE the xbar transpose (half the transpose bytes),
# consumed via DoubleRowSwInterleave. False: baseline-style bf16 transpose
# followed by an fp8 cast, consumed via plain DoubleRow.
PACKED = True

B, T, S, H = 32, 1, 4096, 1024
NCORES = 8
BL = B // NCORES       # batches per core
NS = S // 128          # s-tiles per batch
NHB = H // 128         # h blocks
NKB = 2 * H // 128     # k blocks of cat=[ctx;query]
GT = BL * NS           # global s-tiles per core
NHC = S // 512         # chunks (4 tiles) per batch
GHC = BL * NHC         # global chunks
PREFETCH_HC = 4        # chunks the enc DMA runs ahead
CAST_AHEAD = 8         # tiles the fp8 pack-cast runs ahead
TR_AHEAD = 6           # tiles the transpose runs ahead
CTX_DELAY = 6          # tiles the inline ctx matmuls trail


def _build_program():
    nc = bacc.Bacc("TRN2", target_bir_lowering=False, debug=False)

    q_d = nc.dram_tensor("query", (BL, T, H), F32, kind="ExternalInput").ap()
    enc_d = nc.dram_tensor("encoder_outputs", (BL, S, H), F32,
                           kind="ExternalInput").ap()
    ws_d = nc.dram_tensor("Ws_w", (H, H), F32, kind="ExternalInput").ap()
    wh_d = nc.dram_tensor("Wh_w", (H, H), F32, kind="ExternalInput").ap()
    v_d = nc.dram_tensor("v_w", (1, H), F32, kind="ExternalInput").ap()
    wout_d = nc.dram_tensor("Wout_w", (H, 2 * H), F32, kind="ExternalInput").ap()
    out_d = nc.dram_tensor("out", (BL, T, H), F32, kind="ExternalOutput").ap()
    qp_stash = nc.dram_tensor("qp_stash", (1, BL * H), BF16, kind="Internal").ap()

    with tile.TileContext(nc) as tc, ExitStack() as ctx:
        # ---------------- pools ----------------
        wt_pool = ctx.enter_context(tc.tile_pool(name="wt", bufs=1))
        wnat_pool = ctx.enter_context(tc.tile_pool(name="wnat", bufs=4))
        wonat_pool = ctx.enter_context(tc.tile_pool(name="wonat", bufs=2))
        enc_pool = ctx.enter_context(tc.tile_pool(name="encp", bufs=7))
        pk_pool = ctx.enter_context(tc.tile_pool(name="pkp", bufs=12))
        tr_pool = ctx.enter_context(tc.tile_pool(name="trp", bufs=9))
        sum_pool = ctx.enter_context(tc.tile_pool(name="sump", bufs=3))
        tanh_pool = ctx.enter_context(tc.tile_pool(name="tanhp", bufs=4))
        junk_pool = ctx.enter_context(tc.tile_pool(name="junkp", bufs=1))
        sc_pool = ctx.enter_context(tc.tile_pool(name="scp", bufs=2))
        small_pool = ctx.enter_context(tc.tile_pool(name="smallp", bufs=1))

        hp_psum = ctx.enter_context(tc.tile_pool(name="hp_ps", bufs=3, space="PSUM"))
        ctx_psum = ctx.enter_context(tc.tile_pool(name="ctx_ps", bufs=1, space="PSUM"))
        stg_psum = ctx.enter_context(tc.tile_pool(name="stg_ps", bufs=1, space="PSUM"))
        tr_psum = stg_psum  # prologue transpose staging (1 bank)

        # ---------------- constants ----------------
        id128 = small_pool.tile([128, 128], BF16)
        masks.make_identity(nc, id128[:])
        id4 = small_pool.tile([4, 4], BF16)
        masks.make_identity(nc, id4[:])
        ones_col = small_pool.tile([128, 1], BF16)
        nc.gpsimd.memset(ones_col[:], 1.0)

        # ---------------- encoder pipeline helpers ----------------
        hc_tiles = {}

        def hc_dma(ghc):
            """Load chunk ghc: [128, 4, 1024] bf16 (cast from fp32),
            alternating between the gpsimd and sync DMA queues."""
            if ghc in hc_tiles:
                return hc_tiles[ghc]
            b, hc = divmod(ghc, NHC)
            encN = enc_pool.tile([128, 4, H], BF16, tag="encN")
            src = enc_d[b, hc * 512:(hc + 1) * 512, :]
            nc.gpsimd.dma_start(encN[:], src.rearrange("(t p) h -> p t h", p=128))
            hc_tiles[ghc] = encN
            return encN

        def enc_tile(g):
            return hc_dma(g // 4)[:, g % 4, :]

        pk_tiles = {}
        tr_tiles = {}
        trf_tiles = {}

        if PACKED:
            def cast_tile(g):
                """Contiguous fp8 cast of tile g into a [128, 512] bf16
                container tile (byte x of partition s = fp8(enc[s, h=x]))."""
                if g >= GT or g in pk_tiles:
                    return
                src = enc_tile(g)
                pk = pk_pool.tile([128, 512], BF16, tag="pk", name=f"pk{g}")
                pk8 = pk[:].bitcast(FP8)  # [128, 1024]
                nc.scalar.copy(pk8[:], src)
                pk_tiles[g] = pk

            def tr_tile(g):
                """xbar transpose of the packed tile: [128, 4, 128] bf16.
                fp8 view: byte (2s+r) of partition j, block hb =
                enc[s, h = hb*256 + 2j + r] -> DoubleRow pairs (h, h+256)
                across block pairs at byte step 256."""
                if g >= GT or g in tr_tiles:
                    return
                trt = tr_pool.tile([128, 4, 128], BF16, tag="tr",
                                   name=f"tr{g}")
                nc.sync.dma_start(trt[:], pk_tiles[g][:], transpose=True)
                tr_tiles[g] = trt
        else:
            def cast_tile(g):
                """bf16 xbar transpose of tile g (baseline-style), issued
                here so the pipeline depth matches the packed path."""
                if g >= GT or g in tr_tiles:
                    return
                trt = tr_pool.tile([128, NHB, 128], BF16, tag="tr",
                                   name=f"tr{g}")
                nc.sync.dma_start(trt[:], enc_tile(g), transpose=True)
                tr_tiles[g] = trt

            def tr_tile(g):
                """fp8 cast of the transposed tile (plain DoubleRow
                layout, pair step 128 bytes)."""
                if g >= GT or g in trf_tiles:
                    return
                trf = pk_pool.tile([128, NHB, 128], FP8, tag="pk",
                                   name=f"trf{g}")
                nc.vector.tensor_copy(trf[:, 0:4, :], tr_tiles[g][:, 0:4, :])
                nc.scalar.copy(trf[:, 4:8, :], tr_tiles[g][:, 4:8, :])
                trf_tiles[g] = trf

        # ---------------- weight loads + transposes ----------------
        # enc prefetch first so the DMA engines start on the critical bytes
        for ghc in range(2):
            hc_dma(ghc)
        wh_nat, ws_nat = [], []
        for j in range(NHB):
            wN = wnat_pool.tile([128, H], BF16, tag="wnat")
            nc.gpsimd.dma_start(wN[:], wh_d[j * 128:(j + 1) * 128, :])
            wh_nat.append(wN)
        for ghc in range(2, PREFETCH_HC):
            hc_dma(ghc)

        # whT8 fp8 [128, 8, 1024]. PACKED: slot (2*hb+r) holds WhT rows
        # (hb*256 + 2j + r) at partition j (stride-2 sliced transposes),
        # pairing (h, h+256) to match the packed-transposed encoder.
        # Non-packed: slot hb holds WhT rows (hb*128 + j) (plain).
        whT8 = wt_pool.tile([128, NHB, H], FP8, tag="whT8")
        for jo in range(NHB):
            tp = tr_psum.tile([128, NHB * 128], BF16, tag="hp",
                              name=f"tpwh{jo}")
            if PACKED:
                wsl = wh_nat[jo][:].rearrange("o (x two) -> o two x", two=2)
                for hb in range(4):
                    for r in range(2):
                        nc.tensor.transpose(
                            tp[:, (2 * hb + r) * 128:(2 * hb + r + 1) * 128],
                            wsl[:, r, hb * 128:(hb + 1) * 128],
                            id128[:])
            else:
                for hb in range(NHB):
                    nc.tensor.transpose(tp[:, hb * 128:(hb + 1) * 128],
                                        wh_nat[jo][:, hb * 128:(hb + 1) * 128],
                                        id128[:])
            if jo % 2 == 0:
                nc.vector.tensor_copy(whT8[:, :, jo * 128:(jo + 1) * 128], tp[:])
            else:
                nc.scalar.copy(whT8[:, :, jo * 128:(jo + 1) * 128], tp[:])

        # early casts/transposes for the first tiles
        for g in range(CAST_AHEAD):
            cast_tile(g)
        for g in range(TR_AHEAD):
            tr_tile(g)

        # ---------------- query path (all batches, prologue) ----------
        for j in range(NHB):
            wN = wnat_pool.tile([128, H], BF16, tag="wnat")
            nc.gpsimd.dma_start(wN[:], ws_d[j * 128:(j + 1) * 128, :])
            ws_nat.append(wN)
        q_sb = small_pool.tile([BL, H], BF16)
        nc.gpsimd.dma_start(q_sb[:], q_d[0:BL, 0, :])
        v_row = small_pool.tile([1, H], BF16)
        nc.gpsimd.dma_start(v_row[:], v_d[0:1, :])
        v_bcast = small_pool.tile([128, H], BF16)
        nc.gpsimd.partition_broadcast(v_bcast[:], v_row[:])

        wsT = wt_pool.tile([128, NHB, H], BF16, tag="wsT")
        for j in range(NHB):
            tp = tr_psum.tile([128, NHB * 128], BF16, tag="hp")
            for hb in range(NHB):
                nc.tensor.transpose(tp[:, hb * 128:(hb + 1) * 128],
                                    ws_nat[j][:, hb * 128:(hb + 1) * 128],
                                    id128[:])
            nc.vector.tensor_copy(wsT[:, :, j * 128:(j + 1) * 128], tp[:])

        # catT[k%128, kb, b]: blocks 0..7 = ctx^T (filled at finale),
        # 8..15 = q^T
        catT = small_pool.tile([128, NKB, BL], BF16)
        qt_ps = tr_psum.tile([128, NHB * BL], BF16, tag="hp")
        for j in range(NHB):
            nc.tensor.transpose(qt_ps[:, j * BL:(j + 1) * BL],
                                q_sb[0:BL, j * 128:(j + 1) * 128], id4[:])
        nc.vector.tensor_copy(catT[:, NHB:2 * NHB, :], qt_ps[:])

        qp_ps = hp_psum.tile([BL, H], F32, tag="hp")
        for hb in range(NHB):
            for half in range(2):
                nc.tensor.matmul(qp_ps[:, half * 512:(half + 1) * 512],
                                 catT[:, NHB + hb, :],
                                 wsT[:, hb, half * 512:(half + 1) * 512],
                                 start=(hb == 0), stop=(hb == NHB - 1))
        qp_sb4 = small_pool.tile([BL, H], BF16)
        nc.scalar.copy(qp_sb4[:], qp_ps[:])
        # one DRAM bounce to move each batch's q_proj row to partition 0,
        # then broadcast across partitions (all in the prologue)
        nc.gpsimd.dma_start(qp_stash.rearrange("a (b h) -> (a b) h", b=BL),
                            qp_sb4[:])
        qpb = []
        for b in range(BL):
            qp_row = small_pool.tile([1, H], BF16, tag=f"qprow{b}")
            nc.gpsimd.dma_start(qp_row[:], qp_stash[0:1, b * H:(b + 1) * H])
            qb = small_pool.tile([128, H], BF16, tag=f"qpb{b}")
            nc.gpsimd.partition_broadcast(qb[:], qp_row[:])
            qpb.append(qb)

        ctx4_sb = small_pool.tile([BL, H], BF16)

        # ---------------- per-batch state ----------------
        scores = {}
        attnU = {}
        ctx_ps = {}
        tanh_tiles = {}

        def emit_hp(g):
            hp = hp_psum.tile([128, H], F32, tag="hp", name=f"hp{g}")
            if PACKED:
                # stationary (j, ko, s): byte (2s+r) of block (2hbp+ko) =
                # enc[s, h=(2hbp+ko)*256 + 2j + r]; pair step 256 B
                trv = tr_tiles[g][:].bitcast(FP8).rearrange(
                    "j hb (s two) -> j hb two s", two=2)  # [128,4,2,128]
                whv = whT8[:].rearrange("j (hbp ko r) o -> j hbp r ko o",
                                        hbp=2, ko=2, r=2)
                for hbp in range(2):
                    for r in range(2):
                        stat = trv[:, 2 * hbp:2 * hbp + 2, r, :]
                        for half in range(2):
                            nc.tensor.matmul(
                                hp[:, half * 512:(half + 1) * 512],
                                stat,
                                whv[:, hbp, r, :,
                                    half * 512:(half + 1) * 512],
                                start=(hbp == 0 and r == 0),
                                stop=(hbp == 1 and r == 1),
                                perf_mode=DR)
            else:
                trf = trf_tiles[g]
                for hb2 in range(4):
                    for half in range(2):
                        nc.tensor.matmul(
                            hp[:, half * 512:(half + 1) * 512],
                            trf[:, 2 * hb2:2 * hb2 + 2, :],
                            whT8[:, 2 * hb2:2 * hb2 + 2,
                                 half * 512:(half + 1) * 512],
                            start=(hb2 == 0), stop=(hb2 == 3),
                            perf_mode=DR)
            return hp

        def emit_ctx(g):
            b, st = divmod(g, NS)
            first, last = st == 0, st == NS - 1
            att = attnU[b][:, st:st + 1]
            # denominator at partition 0, ctx halves at partitions 32 / 64
            nc.tensor.matmul(ctx_ps[b][0:1, 0:1], att, ones_col[:],
                             start=first, stop=last)
            for half in range(2):
                nc.tensor.matmul(
                    ctx_ps[b][32 * (half + 1):32 * (half + 1) + 1, :],
                    att, enc_tile(g)[:, half * 512:(half + 1) * 512],
                    start=first, stop=last)

        def emit_dot(g):
            b, st = divmod(g, NS)
            junk = junk_pool.tile([128, H], BF16, tag="junk",
                                  name=f"junk{g}")
            nc.vector.scalar_tensor_tensor(
                out=junk[:], in0=tanh_tiles.pop(g)[:], scalar=1.0,
                in1=v_bcast[:],
                op0=mybir.AluOpType.mult, op1=mybir.AluOpType.mult,
                accum_out=scores[b][:, st:st + 1])

        def emit_exp_pair(g):
            b, st = divmod(g, NS)
            nc.scalar.activation(attnU[b][:, st:st + 2],
                                 scores[b][:, st:st + 2], Exp)

        def emit_denom_a(b):
            inv_d = sc_pool.tile([1, 1], F32, tag="invd", name=f"invd{b}")
            nc.vector.reciprocal(inv_d[:], ctx_ps[b][0:1, 0:1])
            inv_b = sc_pool.tile([128, 1], F32, tag="invb", name=f"invb{b}")
            nc.gpsimd.partition_broadcast(inv_b[:], inv_d[:])
            return inv_b

        def emit_ctx_store(b, inv_b):
            ctx_row = sc_pool.tile([128, 512], BF16, tag="ctxrow",
                                   name=f"ctxrow{b}")
            for half in range(2):
                p = 32 * (half + 1)
                nc.scalar.activation(ctx_row[p:p + 1, :],
                                     ctx_ps[b][p:p + 1, :], Copy,
                                     scale=inv_b[p:p + 1, 0:1])
            # tiny SB->SB partition move on the (serialized) sync queue
            nc.sync.dma_start(ctx4_sb[b:b + 1, :], ctx_row[32:65:32, :])

        inv_ds = {}
        next_ctx = [0]

        # ---------------- main loop over global tiles ----------------
        for g in range(GT):
            b, st = divmod(g, NS)
            if st == 0:
                scores[b] = sc_pool.tile([128, NS], F32, tag="scores", name=f"scores{b}")
                attnU[b] = sc_pool.tile([128, NS], BF16, tag="attnU", name=f"attnU{b}")
                ctx_ps[b] = ctx_psum.tile([128, 512], F32, tag="ctxps",
                                          name=f"ctxps{b}")
            # prefetch / pipeline feeds
            if g % 4 == 0 and g // 4 + PREFETCH_HC < GHC:
                hc_dma(g // 4 + PREFETCH_HC)
            cast_tile(g + CAST_AHEAD)
            tr_tile(g + TR_AHEAD)
            # previous batch's denominator + ctx store (off critical path)
            if b > 0:
                if st == 6:
                    inv_ds[b - 1] = emit_denom_a(b - 1)
                elif st == 7:
                    emit_ctx_store(b - 1, inv_ds.pop(b - 1))

            hp = emit_hp(g)
            # emit pending ctx matmuls; a new batch's ctx is held until
            # st==8 so the previous batch's PSUM bank has been drained
            while next_ctx[0] <= g - CTX_DELAY and next_ctx[0] < GT:
                bc = next_ctx[0] // NS
                if bc > 0 and g < bc * NS + 8:
                    break
                emit_ctx(next_ctx[0])
                next_ctx[0] += 1

            if g >= 2:
                emit_dot(g - 2)
            sum_sb = sum_pool.tile([128, H], BF16, tag="sum",
                                   name=f"sum{g}")
            nc.vector.tensor_add(sum_sb[:], hp[:], qpb[b][:])
            tanh_sb = tanh_pool.tile([128, H], BF16, tag="tanh",
                                     name=f"tanh{g}")
            nc.scalar.activation(tanh_sb[:], sum_sb[:], Tanh)
            tanh_tiles[g] = tanh_sb
            if g >= 4 and g % 2 == 0:
                emit_exp_pair(g - 4)

        # ---------------- drain the pipeline ----------------
        emit_dot(GT - 2)
        emit_dot(GT - 1)
        emit_exp_pair(GT - 4)
        emit_exp_pair(GT - 2)
        while next_ctx[0] < GT:
            emit_ctx(next_ctx[0])
            next_ctx[0] += 1
        inv_ds[BL - 1] = emit_denom_a(BL - 1)
        emit_ctx_store(BL - 1, inv_ds.pop(BL - 1))

        # ---------------- Wout prep (off critical path, xbar) ----------
        woutT = wt_pool.tile([128, NKB, H], BF16, tag="woutT")
        for j in range(NHB):
            with tc.tile_wait_until(0.45 + j * 0.03):
                woN = wonat_pool.tile([128, 2 * H], BF16, tag="wonat")
                nc.gpsimd.dma_start(woN[:], wout_d[j * 128:(j + 1) * 128, :])
            with tc.tile_wait_until(0.52 + j * 0.03):
                nc.sync.dma_start(woutT[:, :, j * 128:(j + 1) * 128], woN[:],
                                  transpose=True)

        # ---------------- finale ----------------
        # query half of the output matmul first (doesn't need ctx)
        out_ps = hp_psum.tile([BL, H], F32, tag="hp")
        with tc.tile_wait_until(0.90):
            for kb in range(NHB, NKB):
                for half in range(2):
                    nc.tensor.matmul(out_ps[:, half * 512:(half + 1) * 512],
                                     catT[:, kb, :],
                                     woutT[:, kb, half * 512:(half + 1) * 512],
                                     start=(kb == NHB), stop=False)
        ct_ps = tr_psum.tile([128, NHB * BL], BF16, tag="hp")
        for j in range(NHB):
            nc.tensor.transpose(ct_ps[:, j * BL:(j + 1) * BL],
                                ctx4_sb[0:BL, j * 128:(j + 1) * 128], id4[:])
        nc.vector.tensor_copy(catT[:, 0:NHB, :], ct_ps[:])
        for kb in range(NHB):
            for half in range(2):
                nc.tensor.matmul(out_ps[:, half * 512:(half + 1) * 512],
                                 catT[:, kb, :],
                                 woutT[:, kb, half * 512:(half + 1) * 512],
                                 start=False, stop=(kb == NHB - 1))
        out_sb = small_pool.tile([BL, H], BF16)
        nc.scalar.activation(out_sb[:], out_ps[:], Tanh)
        nc.gpsimd.dma_start(out_d[0:BL, 0, :], out_sb[:])

    nc.compile()
    return nc


_program = None


def get_program():
    global _program
    if _program is None:
        _program = _build_program()
    return _program


def run_sharded(inputs, trace=False, **kw):
    nc = get_program()
    in_maps = []
    for i in range(NCORES):
        sl = slice(i * BL, (i + 1) * BL)
        in_maps.append({
            "query": np.ascontiguousarray(inputs["query"][sl], dtype=np.float32),
            "encoder_outputs": np.ascontiguousarray(
                inputs["encoder_outputs"][sl], dtype=np.float32),
            "Ws_w": np.asarray(inputs["Ws_w"], dtype=np.float32),
            "Wh_w": np.asarray(inputs["Wh_w"], dtype=np.float32),
            "v_w": np.asarray(inputs["v_w"], dtype=np.float32),
            "Wout_w": np.asarray(inputs["Wout_w"], dtype=np.float32),
        })
    res = bass_utils.run_bass_kernel_spmd(
        nc, in_maps, core_ids=list(range(NCORES)), trace=trace, **kw)
    out = np.concatenate(
        [np.asarray(res.results[i]["out"], dtype=np.float32).reshape(BL, T, H)
         for i in range(NCORES)], axis=0)
    return out, res


def kernel(**inputs):
    out, _ = run_sharded(inputs)
    return out
